# revision 5
# baseline (speedup 1.0000x reference)
"""Multi-head causal attention (RoPE) on 8 Trainium2 NeuronCores.

Sharding (Megatron-style): core c handles batch c//4 and the 4 heads
[4*(c%4), 4*(c%4)+4). Each core computes Q/K/V projections for its
head slice, rotary embedding, causal flash-style attention (no
max-subtraction: scores are O(10) so exp is safe in f32), and its
partial output projection through the matching Wo column block. The
host sums the 4 partial outputs per batch and transposes (the device
computes out.T: [model_dim, seq]).

All on-device layouts are transposed ([feature, seq]) so that
- projections use hsT tiles as the moving operand (N=512 matmuls),
- QK^T produces scores.T directly ([key, query]) which is what the
  AV matmul wants as its moving operand, and
- softmax normalization uses a ones-matmul partition-sum that also
  broadcasts the denominator across partitions.
Matmul inputs are bf16 (f32 PE matmul is 2x slower); accumulation is
always f32 in PSUM.
"""

import os

import numpy as np
import ml_dtypes

import concourse.bass as bass
import concourse.mybir as mybir
import concourse.tile as tile
from concourse import bacc
from concourse.bass_utils import run_bass_kernel_spmd

BF16 = mybir.dt.bfloat16
F32 = mybir.dt.float32
NPBF16 = ml_dtypes.bfloat16

NCORES = 8
B = 2
S = 2048
HDIM = 2048
NH = 16
HD = 128
HPC = 4  # heads per core
CPB = 4  # cores per batch
SCW = 512  # s-chunk width
NSC = S // SCW  # 4
KT = HDIM // 128  # 16 k-tiles
NJT = S // 128  # 16 j-tiles
SCALE = 1.0 / np.sqrt(HD)
ROPE_BASE = 10000.0

_NC_CACHE: dict[str, object] = {}
LAST_EXEC_TIME_NS = None


def _build(mode: str):
    """mode: 'causal' | 'full' | 'general'"""
    nc = bacc.Bacc("TRN2", target_bir_lowering=False, debug=False,
                   num_devices=NCORES)

    hst_d = nc.declare_dram_parameter("hst", [NSC, 128, KT * SCW], BF16, isOutput=False)
    wq_d = nc.declare_dram_parameter("wq", [128, HPC * KT * 128], BF16, isOutput=False)
    wk_d = nc.declare_dram_parameter("wk", [128, HPC * KT * 128], BF16, isOutput=False)
    wv_d = nc.declare_dram_parameter("wv", [128, KT * 512], BF16, isOutput=False)
    wo_d = nc.declare_dram_parameter("wo", [128, HPC * KT * 128], BF16, isOutput=False)
    cos_d = nc.declare_dram_parameter("cost", [64, S], F32, isOutput=False)
    sin_d = nc.declare_dram_parameter("sint", [64, S], F32, isOutput=False)
    bm_d = nc.declare_dram_parameter("bmask", [128, 896], BF16, isOutput=False)
    if mode == "general":
        em_d = nc.declare_dram_parameter("emask", [S, S], BF16, isOutput=False)
    out_d = nc.declare_dram_parameter("outT", [HDIM, S], F32, isOutput=True)

    Exp = mybir.ActivationFunctionType.Exp

    with tile.TileContext(nc) as tc:
        with (
            tc.tile_pool(name="wpool", bufs=1) as wpool,
            tc.tile_pool(name="cpool", bufs=1) as cpool,
            tc.tile_pool(name="qkv", bufs=1) as qkvp,
            tc.tile_pool(name="strip", bufs=2) as stripp,
            tc.tile_pool(name="ropet", bufs=2) as ropet,
            tc.tile_pool(name="probs", bufs=4) as probsp,
            tc.tile_pool(name="psums", bufs=2) as psums,
            tc.tile_pool(name="recips", bufs=2) as recips,
            tc.tile_pool(name="fouts", bufs=3) as fouts,
            tc.tile_pool(name="emt", bufs=4) as emtp,
            tc.tile_pool(name="pp", bufs=2, space="PSUM") as pp_pool,
            tc.tile_pool(name="sp", bufs=2, space="PSUM") as sp_pool,
            tc.tile_pool(name="av", bufs=2, space="PSUM") as av_pool,
            tc.tile_pool(name="misc_ps", bufs=2, space="PSUM") as misc_ps,
        ):
            wq = wpool.tile([128, HPC * KT * 128], BF16, tag="wq")
            nc.sync.dma_start(wq[:], wq_d[:])
            wk = wpool.tile([128, HPC * KT * 128], BF16, tag="wk")
            nc.sync.dma_start(wk[:], wk_d[:])
            wv = wpool.tile([128, KT * 512], BF16, tag="wv")
            nc.sync.dma_start(wv[:], wv_d[:])
            wo = wpool.tile([128, HPC * KT * 128], BF16, tag="wo")
            nc.sync.dma_start(wo[:], wo_d[:])
            cos = cpool.tile([64, S], F32, tag="cos")
            nc.sync.dma_start(cos[:], cos_d[:])
            sin = cpool.tile([64, S], F32, tag="sin")
            nc.sync.dma_start(sin[:], sin_d[:])
            bm = cpool.tile([128, 896], BF16, tag="bm")
            nc.sync.dma_start(bm[:], bm_d[:])
            ones = cpool.tile([128, 128], F32, tag="ones")
            nc.gpsimd.memset(ones[:], 1.0)

            qT = qkvp.tile([128, HPC * S], BF16, tag="qT")
            kTt = qkvp.tile([128, HPC * S], BF16, tag="kT")
            vN = qkvp.tile([128, NJT * 512], BF16, tag="vN")
            oT = qkvp.tile([128, HPC * NSC * 512], BF16, tag="oT")

            # ---- Phase A: projections + rope, one 512-col s-strip at a time
            for sc in range(NSC):
                strip = stripp.tile([128, KT * SCW], BF16)
                nc.sync.dma_start(strip[:], hst_d[sc])
                cs = cos[:, sc * SCW:(sc + 1) * SCW]
                sn = sin[:, sc * SCW:(sc + 1) * SCW]
                for h in range(HPC):
                    for wt, dst in ((wq, qT), (wk, kTt)):
                        pq = pp_pool.tile([128, SCW], F32, tag="pp")
                        for kt in range(KT):
                            nc.tensor.matmul(
                                pq[:],
                                wt[:, (h * KT + kt) * 128:(h * KT + kt + 1) * 128],
                                strip[:, kt * SCW:(kt + 1) * SCW],
                                start=(kt == 0), stop=(kt == KT - 1),
                            )
                        # rope: rotate_half via partition-base-shifted PSUM
                        # reads (walrus only requires equal base partitions
                        # when BOTH inputs are SBUF, so the shifted reads go
                        # through the PSUM operand).
                        # A = [q_lo*cos ; q_hi*cos], R = [q_hi*sin ; q_lo*sin]
                        dlo = dst[0:64, h * S + sc * SCW: h * S + (sc + 1) * SCW]
                        dhi = dst[64:128, h * S + sc * SCW: h * S + (sc + 1) * SCW]
                        t1 = ropet.tile([128, SCW], F32, tag="t1")
                        t2 = ropet.tile([128, SCW], F32, tag="t2")
                        nc.vector.tensor_mul(t1[0:64, :], pq[0:64, :], cs)
                        nc.vector.tensor_mul(t1[64:128, :], pq[64:128, :], cs)
                        nc.vector.tensor_mul(t2[0:64, :], pq[64:128, :], sn)
                        nc.vector.tensor_mul(t2[64:128, :], pq[0:64, :], sn)
                        nc.vector.tensor_sub(dlo, t1[0:64, :], t2[0:64, :])
                        nc.vector.tensor_add(dhi, t1[64:128, :], t2[64:128, :])
                for st in range(4):
                    vp = pp_pool.tile([128, SCW], F32, tag="pp")
                    for kt in range(KT):
                        nc.tensor.matmul(
                            vp[:],
                            strip[:, kt * SCW + st * 128: kt * SCW + (st + 1) * 128],
                            wv[:, kt * 512:(kt + 1) * 512],
                            start=(kt == 0), stop=(kt == KT - 1),
                        )
                    jt = sc * 4 + st
                    nc.scalar.copy(vN[:, jt * 512:(jt + 1) * 512], vp[:])

            # ---- Phase B: attention per (i-chunk, head); Phase C: Wo per i-chunk
            for ic in range(NSC):
                nj = 4 * (ic + 1) if mode == "causal" else NJT
                for h in range(HPC):
                    av = av_pool.tile([128, SCW], F32)
                    Ps = psums.tile([128, SCW], F32)
                    for j in range(nj):
                        sp = sp_pool.tile([128, SCW], F32)
                        nc.tensor.matmul(
                            sp[:],
                            kTt[:, h * S + j * 128: h * S + (j + 1) * 128],
                            qT[:, h * S + ic * SCW: h * S + (ic + 1) * SCW],
                            start=True, stop=True,
                        )
                        pr = probsp.tile([128, SCW], BF16)
                        nc.scalar.activation(pr[:], sp[:], Exp, scale=float(SCALE))
                        if mode == "causal" and j >= ic * 4:
                            c0 = 384 + ic * SCW - j * 128
                            nc.vector.tensor_mul(pr[:], pr[:], bm[:, c0:c0 + SCW])
                        elif mode == "general":
                            emt = emtp.tile([128, SCW], BF16)
                            nc.sync.dma_start(
                                emt[:],
                                em_d[j * 128:(j + 1) * 128, ic * SCW:(ic + 1) * SCW],
                            )
                            nc.vector.tensor_mul(pr[:], pr[:], emt[:])
                        if j == 0:
                            nc.vector.tensor_copy(Ps[:], pr[:])
                        else:
                            nc.vector.tensor_add(Ps[:], Ps[:], pr[:])
                        nc.tensor.matmul(
                            av[:],
                            vN[:, j * 512 + h * 128: j * 512 + (h + 1) * 128],
                            pr[:],
                            start=(j == 0), stop=(j == nj - 1),
                        )
                    rs = misc_ps.tile([128, SCW], F32, tag="mp")
                    nc.tensor.matmul(rs[:], ones[:], Ps[:], start=True, stop=True)
                    rc = recips.tile([128, SCW], F32)
                    nc.vector.reciprocal(rc[:], rs[:])
                    nc.vector.tensor_mul(
                        oT[:, (h * NSC + ic) * 512:(h * NSC + ic + 1) * 512],
                        av[:], rc[:],
                    )
                for mt in range(KT):
                    fp = misc_ps.tile([128, SCW], F32, tag="mp")
                    for h in range(HPC):
                        nc.tensor.matmul(
                            fp[:],
                            wo[:, (h * KT + mt) * 128:(h * KT + mt + 1) * 128],
                            oT[:, (h * NSC + ic) * 512:(h * NSC + ic + 1) * 512],
                            start=(h == 0), stop=(h == HPC - 1),
                        )
                    fs = fouts.tile([128, SCW], F32)
                    nc.scalar.copy(fs[:], fp[:])
                    nc.sync.dma_start(
                        out_d[mt * 128:(mt + 1) * 128, ic * SCW:(ic + 1) * SCW],
                        fs[:],
                    )

    nc.compile()
    return nc


def _get_nc(mode: str):
    if mode not in _NC_CACHE:
        _NC_CACHE[mode] = _build(mode)
    return _NC_CACHE[mode]


def _classify_mask(m: np.ndarray) -> str:
    if not m.any():
        return "full"
    tril = np.tril(np.ones((S, S), dtype=bool))
    if np.all(m[tril] == 0.0) and np.all(m[~tril] <= -1e8):
        return "causal"
    return "general"


def kernel(hidden_states, attention_mask, position_ids, Wq, Wk, Wv, Wo):
    global LAST_EXEC_TIME_NS
    hs = np.asarray(hidden_states, dtype=np.float32)
    mask = np.asarray(attention_mask, dtype=np.float32)[0, 0]
    pos = np.asarray(position_ids)
    Wq = np.asarray(Wq, dtype=np.float32)
    Wk = np.asarray(Wk, dtype=np.float32)
    Wv = np.asarray(Wv, dtype=np.float32)
    Wo = np.asarray(Wo, dtype=np.float32)

    mode = _classify_mask(mask)
    nc = _get_nc(mode)

    # rope tables per batch: [64, S] (emb = concat(freqs, freqs) -> the
    # two halves of the head dim share the same angle table)
    inv_freq = 1.0 / (ROPE_BASE ** (np.arange(0, HD, 2, dtype=np.float32) / HD))
    cos_b, sin_b = [], []
    for b in range(B):
        ang = np.outer(pos[b].astype(np.float32), inv_freq)  # [S, 64]
        cos_b.append(np.cos(ang).T.astype(np.float32).copy())
        sin_b.append(np.sin(ang).T.astype(np.float32).copy())

    # band mask [128, 896]: bm[p, x] = 1 if p <= x - 384
    pidx = np.arange(128)[:, None]
    xidx = np.arange(896)[None, :]
    bmask = (pidx <= xidx - 384).astype(NPBF16)

    emask = None
    if mode == "general":
        with np.errstate(under="ignore", over="ignore"):
            emask = np.exp(mask.T.astype(np.float64)).astype(NPBF16)

    in_maps = []
    for c in range(NCORES):
        b = c // CPB
        r0 = (c % CPB) * HPC * HD  # feature-row base of this core's heads

        hsb = hs[b]  # [S, HDIM]
        hst = (hsb.reshape(NSC, SCW, KT, 128).transpose(0, 3, 2, 1)
               .reshape(NSC, 128, KT * SCW).astype(NPBF16))

        Wq_s = Wq[r0:r0 + 512]  # [512, HDIM]
        wq_t = (Wq_s.reshape(HPC, 128, KT, 128).transpose(3, 0, 2, 1)
                .reshape(128, HPC * KT * 128).astype(NPBF16))
        Wk_s = Wk[r0:r0 + 512]
        wk_t = (Wk_s.reshape(HPC, 128, KT, 128).transpose(3, 0, 2, 1)
                .reshape(128, HPC * KT * 128).astype(NPBF16))
        Wv_s = Wv[r0:r0 + 512]  # [512, HDIM]
        wv_t = (Wv_s.reshape(512, KT, 128).transpose(2, 1, 0)
                .reshape(128, KT * 512).astype(NPBF16))
        Wo_s = Wo[:, r0:r0 + 512]  # [HDIM, 512]
        wo_t = (Wo_s.reshape(KT, 128, HPC, 128).transpose(3, 2, 0, 1)
                .reshape(128, HPC * KT * 128).astype(NPBF16))

        m = {
            "hst": hst, "wq": wq_t, "wk": wk_t, "wv": wv_t, "wo": wo_t,
            "cost": cos_b[b], "sint": sin_b[b], "bmask": bmask,
        }
        if mode == "general":
            m["emask"] = emask
        in_maps.append(m)

    trace = os.environ.get("BASS_KERNEL_TRACE") == "1"
    res = run_bass_kernel_spmd(nc, in_maps, core_ids=list(range(NCORES)),
                               trace=trace)
    LAST_EXEC_TIME_NS = res.exec_time_ns

    out = np.empty((B, S, HDIM), dtype=np.float32)
    for b in range(B):
        acc = res.results[CPB * b]["outT"].astype(np.float32)
        for c in range(CPB * b + 1, CPB * (b + 1)):
            acc = acc + res.results[c]["outT"]
        out[b] = acc.T
    return out


# revision 7
# speedup vs baseline: 1.1410x; 1.1410x over previous
"""Multi-head causal attention (RoPE) on 8 Trainium2 NeuronCores.

Sharding (Megatron-style): core c handles batch c//4 and the 4 heads
[4*(c%4), 4*(c%4)+4). Each core computes Q/K/V projections for its
head slice, rotary embedding, causal flash-style attention (no
max-subtraction: scores are O(10) so exp is safe in f32), and its
partial output projection through the matching Wo column block. The
host sums the 4 partial outputs per batch and transposes (the device
computes out.T: [model_dim, seq]).

All on-device layouts are transposed ([feature, seq]) so that
- projections use hsT tiles as the moving operand (N=512 matmuls),
- QK^T produces scores.T directly ([key, query]) which is what the
  AV matmul wants as its moving operand, and
- softmax normalization uses a ones-matmul partition-sum that also
  broadcasts the denominator across partitions.
Matmul inputs are bf16 (f32 PE matmul is 2x slower); accumulation is
always f32 in PSUM.
"""

import os

import numpy as np
import ml_dtypes

import concourse.bass as bass
import concourse.mybir as mybir
import concourse.tile as tile
from concourse import bacc
from concourse.bass_utils import run_bass_kernel_spmd

BF16 = mybir.dt.bfloat16
F16 = mybir.dt.float16
F32 = mybir.dt.float32
NPBF16 = ml_dtypes.bfloat16
NPF16 = np.float16

NCORES = 8
B = 2
S = 2048
HDIM = 2048
NH = 16
HD = 128
HPC = 4  # heads per core
CPB = 4  # cores per batch
SCW = 512  # s-chunk width
NSC = S // SCW  # 4
KT = HDIM // 128  # 16 k-tiles
NJT = S // 128  # 16 j-tiles
SCALE = 1.0 / np.sqrt(HD)
ROPE_BASE = 10000.0

_NC_CACHE: dict[str, object] = {}
LAST_EXEC_TIME_NS = None


def _build(mode: str):
    """mode: 'causal' | 'full' | 'general'"""
    nc = bacc.Bacc("TRN2", target_bir_lowering=False, debug=False,
                   num_devices=NCORES)

    hst_d = nc.declare_dram_parameter("hst", [NSC, 128, KT * SCW], BF16, isOutput=False)
    wq_d = nc.declare_dram_parameter("wq", [128, HPC * KT * 128], BF16, isOutput=False)
    wk_d = nc.declare_dram_parameter("wk", [128, HPC * KT * 128], BF16, isOutput=False)
    wv_d = nc.declare_dram_parameter("wv", [128, KT * 512], BF16, isOutput=False)
    wo_d = nc.declare_dram_parameter("wo", [128, HPC * KT * 128], BF16, isOutput=False)
    cos_d = nc.declare_dram_parameter("cost", [64, S], F32, isOutput=False)
    sin_d = nc.declare_dram_parameter("sint", [64, S], F32, isOutput=False)
    bm_d = nc.declare_dram_parameter("bmask", [128, 896], F16, isOutput=False)
    if mode == "general":
        em_d = nc.declare_dram_parameter("emask", [S, S], F16, isOutput=False)
    out_d = nc.declare_dram_parameter("outT", [HDIM, S], F32, isOutput=True)

    Exp = mybir.ActivationFunctionType.Exp

    with tile.TileContext(nc) as tc:
        with (
            tc.tile_pool(name="wpool", bufs=1) as wpool,
            tc.tile_pool(name="cpool", bufs=1) as cpool,
            tc.tile_pool(name="qkv", bufs=1) as qkvp,
            tc.tile_pool(name="strip", bufs=2) as stripp,
            tc.tile_pool(name="ropet", bufs=2) as ropet,
            tc.tile_pool(name="probs", bufs=4) as probsp,
            tc.tile_pool(name="psums", bufs=2) as psums,
            tc.tile_pool(name="recips", bufs=2) as recips,
            tc.tile_pool(name="fouts", bufs=3) as fouts,
            tc.tile_pool(name="emt", bufs=4) as emtp,
            tc.tile_pool(name="pp", bufs=2, space="PSUM") as pp_pool,
            tc.tile_pool(name="sp", bufs=2, space="PSUM") as sp_pool,
            tc.tile_pool(name="av", bufs=2, space="PSUM") as av_pool,
            tc.tile_pool(name="misc_ps", bufs=2, space="PSUM") as misc_ps,
        ):
            wq = wpool.tile([128, HPC * KT * 128], BF16, tag="wq")
            nc.sync.dma_start(wq[:], wq_d[:])
            wk = wpool.tile([128, HPC * KT * 128], BF16, tag="wk")
            nc.sync.dma_start(wk[:], wk_d[:])
            wv = wpool.tile([128, KT * 512], BF16, tag="wv")
            nc.sync.dma_start(wv[:], wv_d[:])
            wo = wpool.tile([128, HPC * KT * 128], BF16, tag="wo")
            nc.sync.dma_start(wo[:], wo_d[:])
            cos = cpool.tile([64, S], F32, tag="cos")
            nc.sync.dma_start(cos[:], cos_d[:])
            sin = cpool.tile([64, S], F32, tag="sin")
            nc.sync.dma_start(sin[:], sin_d[:])
            bm = cpool.tile([128, 896], F16, tag="bm")
            nc.sync.dma_start(bm[:], bm_d[:])
            ones = cpool.tile([128, 128], F16, tag="ones")
            nc.gpsimd.memset(ones[:], 1.0)

            qT = qkvp.tile([128, HPC * S], BF16, tag="qT")
            kTt = qkvp.tile([128, HPC * S], BF16, tag="kT")
            vN = qkvp.tile([128, NJT * 512], F16, tag="vN")
            oT = qkvp.tile([128, HPC * NSC * 512], BF16, tag="oT")

            # ---- Phase A: projections + rope, one 512-col s-strip at a time
            for sc in range(NSC):
                strip = stripp.tile([128, KT * SCW], BF16)
                nc.sync.dma_start(strip[:], hst_d[sc])
                cs = cos[:, sc * SCW:(sc + 1) * SCW]
                sn = sin[:, sc * SCW:(sc + 1) * SCW]
                for h in range(HPC):
                    for wt, dst in ((wq, qT), (wk, kTt)):
                        pq = pp_pool.tile([128, SCW], F32, tag="pp")
                        for kt in range(KT):
                            nc.tensor.matmul(
                                pq[:],
                                wt[:, (h * KT + kt) * 128:(h * KT + kt + 1) * 128],
                                strip[:, kt * SCW:(kt + 1) * SCW],
                                start=(kt == 0), stop=(kt == KT - 1),
                            )
                        # rope: rotate_half via partition-base-shifted PSUM
                        # reads (walrus only requires equal base partitions
                        # when BOTH inputs are SBUF, so the shifted reads go
                        # through the PSUM operand).
                        # A = [q_lo*cos ; q_hi*cos], R = [q_hi*sin ; q_lo*sin]
                        dlo = dst[0:64, h * S + sc * SCW: h * S + (sc + 1) * SCW]
                        dhi = dst[64:128, h * S + sc * SCW: h * S + (sc + 1) * SCW]
                        t1 = ropet.tile([128, SCW], F32, tag="t1")
                        t2 = ropet.tile([128, SCW], F32, tag="t2")
                        nc.vector.tensor_mul(t1[0:64, :], pq[0:64, :], cs)
                        nc.vector.tensor_mul(t1[64:128, :], pq[64:128, :], cs)
                        nc.vector.tensor_mul(t2[0:64, :], pq[64:128, :], sn)
                        nc.vector.tensor_mul(t2[64:128, :], pq[0:64, :], sn)
                        nc.vector.tensor_sub(dlo, t1[0:64, :], t2[0:64, :])
                        nc.vector.tensor_add(dhi, t1[64:128, :], t2[64:128, :])
                for st in range(4):
                    vp = pp_pool.tile([128, SCW], F32, tag="pp")
                    for kt in range(KT):
                        nc.tensor.matmul(
                            vp[:],
                            strip[:, kt * SCW + st * 128: kt * SCW + (st + 1) * 128],
                            wv[:, kt * 512:(kt + 1) * 512],
                            start=(kt == 0), stop=(kt == KT - 1),
                        )
                    jt = sc * 4 + st
                    nc.scalar.copy(vN[:, jt * 512:(jt + 1) * 512], vp[:])

            # ---- Phase B: attention per (i-chunk, head); Phase C: Wo per i-chunk
            for ic in range(NSC):
                nj = 4 * (ic + 1) if mode == "causal" else NJT
                for h in range(HPC):
                    av = av_pool.tile([128, SCW], F32)
                    Ps = psums.tile([128, SCW], F16)
                    for j in range(nj):
                        sp = sp_pool.tile([128, SCW], F32)
                        nc.tensor.matmul(
                            sp[:],
                            kTt[:, h * S + j * 128: h * S + (j + 1) * 128],
                            qT[:, h * S + ic * SCW: h * S + (ic + 1) * SCW],
                            start=True, stop=True,
                        )
                        pr = probsp.tile([128, SCW], F16)
                        nc.scalar.activation(pr[:], sp[:], Exp, scale=float(SCALE))
                        if mode == "causal" and j >= ic * 4:
                            c0 = 384 + ic * SCW - j * 128
                            nc.vector.tensor_mul(pr[:], pr[:], bm[:, c0:c0 + SCW])
                        elif mode == "general":
                            emt = emtp.tile([128, SCW], F16)
                            nc.sync.dma_start(
                                emt[:],
                                em_d[j * 128:(j + 1) * 128, ic * SCW:(ic + 1) * SCW],
                            )
                            nc.vector.tensor_mul(pr[:], pr[:], emt[:])
                        if j == 0:
                            nc.vector.tensor_copy(Ps[:], pr[:])
                        else:
                            nc.vector.tensor_add(Ps[:], Ps[:], pr[:])
                        nc.tensor.matmul(
                            av[:],
                            vN[:, j * 512 + h * 128: j * 512 + (h + 1) * 128],
                            pr[:],
                            start=(j == 0), stop=(j == nj - 1),
                        )
                    rs = misc_ps.tile([128, SCW], F32, tag="mp")
                    nc.tensor.matmul(rs[:], ones[:], Ps[:], start=True, stop=True)
                    rc = recips.tile([128, SCW], F32)
                    nc.vector.reciprocal_approx_fast(rc[:], rs[:])
                    nc.vector.tensor_mul(
                        oT[:, (h * NSC + ic) * 512:(h * NSC + ic + 1) * 512],
                        av[:], rc[:],
                    )
                for mt in range(KT):
                    fp = misc_ps.tile([128, SCW], F32, tag="mp")
                    for h in range(HPC):
                        nc.tensor.matmul(
                            fp[:],
                            wo[:, (h * KT + mt) * 128:(h * KT + mt + 1) * 128],
                            oT[:, (h * NSC + ic) * 512:(h * NSC + ic + 1) * 512],
                            start=(h == 0), stop=(h == HPC - 1),
                        )
                    fs = fouts.tile([128, SCW], F32)
                    nc.scalar.copy(fs[:], fp[:])
                    nc.sync.dma_start(
                        out_d[mt * 128:(mt + 1) * 128, ic * SCW:(ic + 1) * SCW],
                        fs[:],
                    )

    nc.compile()
    return nc


def _get_nc(mode: str):
    if mode not in _NC_CACHE:
        _NC_CACHE[mode] = _build(mode)
    return _NC_CACHE[mode]


def _classify_mask(m: np.ndarray) -> str:
    if not m.any():
        return "full"
    tril = np.tril(np.ones((S, S), dtype=bool))
    if np.all(m[tril] == 0.0) and np.all(m[~tril] <= -1e8):
        return "causal"
    return "general"


def kernel(hidden_states, attention_mask, position_ids, Wq, Wk, Wv, Wo):
    global LAST_EXEC_TIME_NS
    hs = np.asarray(hidden_states, dtype=np.float32)
    mask = np.asarray(attention_mask, dtype=np.float32)[0, 0]
    pos = np.asarray(position_ids)
    Wq = np.asarray(Wq, dtype=np.float32)
    Wk = np.asarray(Wk, dtype=np.float32)
    Wv = np.asarray(Wv, dtype=np.float32)
    Wo = np.asarray(Wo, dtype=np.float32)

    mode = _classify_mask(mask)
    nc = _get_nc(mode)

    # rope tables per batch: [64, S] (emb = concat(freqs, freqs) -> the
    # two halves of the head dim share the same angle table)
    inv_freq = 1.0 / (ROPE_BASE ** (np.arange(0, HD, 2, dtype=np.float32) / HD))
    cos_b, sin_b = [], []
    for b in range(B):
        ang = np.outer(pos[b].astype(np.float32), inv_freq)  # [S, 64]
        cos_b.append(np.cos(ang).T.astype(np.float32).copy())
        sin_b.append(np.sin(ang).T.astype(np.float32).copy())

    # band mask [128, 896]: bm[p, x] = 1 if p <= x - 384
    pidx = np.arange(128)[:, None]
    xidx = np.arange(896)[None, :]
    bmask = (pidx <= xidx - 384).astype(NPF16)

    emask = None
    if mode == "general":
        with np.errstate(under="ignore", over="ignore"):
            emask = np.exp(mask.T.astype(np.float64)).astype(NPF16)

    in_maps = []
    for c in range(NCORES):
        b = c // CPB
        r0 = (c % CPB) * HPC * HD  # feature-row base of this core's heads

        hsb = hs[b]  # [S, HDIM]
        hst = (hsb.reshape(NSC, SCW, KT, 128).transpose(0, 3, 2, 1)
               .reshape(NSC, 128, KT * SCW).astype(NPBF16))

        Wq_s = Wq[r0:r0 + 512]  # [512, HDIM]
        wq_t = (Wq_s.reshape(HPC, 128, KT, 128).transpose(3, 0, 2, 1)
                .reshape(128, HPC * KT * 128).astype(NPBF16))
        Wk_s = Wk[r0:r0 + 512]
        wk_t = (Wk_s.reshape(HPC, 128, KT, 128).transpose(3, 0, 2, 1)
                .reshape(128, HPC * KT * 128).astype(NPBF16))
        Wv_s = Wv[r0:r0 + 512]  # [512, HDIM]
        wv_t = (Wv_s.reshape(512, KT, 128).transpose(2, 1, 0)
                .reshape(128, KT * 512).astype(NPBF16))
        Wo_s = Wo[:, r0:r0 + 512]  # [HDIM, 512]
        wo_t = (Wo_s.reshape(KT, 128, HPC, 128).transpose(3, 2, 0, 1)
                .reshape(128, HPC * KT * 128).astype(NPBF16))

        m = {
            "hst": hst, "wq": wq_t, "wk": wk_t, "wv": wv_t, "wo": wo_t,
            "cost": cos_b[b], "sint": sin_b[b], "bmask": bmask,
        }
        if mode == "general":
            m["emask"] = emask
        in_maps.append(m)

    trace = os.environ.get("BASS_KERNEL_TRACE") == "1"
    res = run_bass_kernel_spmd(nc, in_maps, core_ids=list(range(NCORES)),
                               trace=trace)
    LAST_EXEC_TIME_NS = res.exec_time_ns

    out = np.empty((B, S, HDIM), dtype=np.float32)
    for b in range(B):
        acc = res.results[CPB * b]["outT"].astype(np.float32)
        for c in range(CPB * b + 1, CPB * (b + 1)):
            acc = acc + res.results[c]["outT"]
        out[b] = acc.T
    return out
